# revision 22
# baseline (speedup 1.0000x reference)
"""Trainium2 Bass kernel for a dense transformer block (B=4, T=1024, C=1024, H=16).

Sharding: 8 cores = 4 batches x 2 tensor-parallel groups (8 heads + half the
FFN hidden dim per core).

Phase A (attention, per core): takes h = LN1(x) (computed on host) in fp8
[C, T] layout.  On chip: qk/v head projections (fp8 DoubleRow matmuls,
K=256/instr), causal scores (bf16, two heads packed in the PE array via
tile_position), exp with a constant -3 offset folded in (keeps e4m3 in range;
cancels in the softmax ratio), and p@[v|1] fp8 DoubleRow matmuls whose extra
ones-column yields the softmax denominator for free.  Ships unnormalized
[attn ; den] per (head, t-chunk) to the host, which divides, applies the
output projection Wp, and forms x2 = x + attn@Wp^T + bp.  Score chunks are
round-robin interleaved with the other matmuls so the PE keeps streaming
while the Scalar engine (exp) drains behind it.

Phase B (FFN, per core): takes h2 = LN2(x2) (host) in fp8.  FFN1 (fp8
DoubleRow) -> ReLU (descale 2^-8 folded into the activation) -> FFN2 (fp8
DoubleRow) -> partial [C, T] output; host descales 2^-9, adds residual + b2.

Weights are pre-scaled by 2^8 (Wk, Wv, W1) / 2^9 (W2) on the host so their
uniform(-1/32..) ranges land in e4m3's normal range; all scales are powers of
two (exact) and are undone on-chip (ReLU) or on the host.
"""
import sys

sys.path.insert(0, "/opt/trn_rl_repo")

import numpy as np
import ml_dtypes
from contextlib import ExitStack

import concourse.bacc as bacc
import concourse.mybir as mybir
import concourse.tile as tile

bf16 = mybir.dt.bfloat16
f32 = mybir.dt.float32
fp8 = mybir.dt.float8e4
np_fp8 = ml_dtypes.float8_e4m3
DR = mybir.MatmulPerfMode.DoubleRow

B, T, C, H = 4, 1024, 1024, 16
HD = 64                    # head dim
NHG = 8                    # heads per core (group)
DG = NHG * HD              # 512, channel span per head group
F = 4 * C                  # 4096 FFN hidden
FG = F // 2                # 2048 per core
P = 128                    # partitions
EPS = 1e-5
SCALE = HD ** -0.5         # 0.125
EOFF = -3.0                # constant exp offset; cancels in softmax ratio

NT = T // P                # 8 c-tiles of 128
TCH = 512                  # t-chunk
NTC = T // TCH             # 2 t-chunks
NF = FG // P               # 16 hidden tiles per core
VW = 80                    # per-head stride in v_aug (65 used, padded to 80 for
                           # 16B-aligned DoubleRow slab steps)

WS1 = 2.0 ** 8             # host pre-scale on Wk/Wv/W1
WS2 = 2.0 ** 9             # host pre-scale on W2
Exp = mybir.ActivationFunctionType.Exp
Relu = mybir.ActivationFunctionType.Relu


def build_phase_a():
    nc = bacc.Bacc("TRN2", target_bir_lowering=False, debug=False)
    hTd = nc.dram_tensor("hT", [P, NT * T], fp8, kind="ExternalInput")
    wkd = nc.dram_tensor("wk", [P, NT * DG], fp8, kind="ExternalInput")
    wvd = nc.dram_tensor("wv", [P, NT * DG], fp8, kind="ExternalInput")
    trid = nc.dram_tensor("tri", [P, 2 * P], fp8, kind="ExternalInput")
    # av: per (pr, tj): rows [65*pr, 65*pr+65), cols [1024*tj + 512*k + t]
    # rows 0-63 = unnormalized attn dims of head 2*pr+k, row 64 = denominator
    avd = nc.dram_tensor("av", [4 * 65, 2 * T], bf16, kind="ExternalOutput")

    with tile.TileContext(nc) as tc, ExitStack() as ctx:
        persist = ctx.enter_context(tc.tile_pool(name="persist", bufs=1))
        psum = ctx.enter_context(tc.tile_pool(name="psum", bufs=1, space="PSUM"))
        ppool = ctx.enter_context(tc.tile_pool(name="ppool", bufs=2))
        opool = ctx.enter_context(tc.tile_pool(name="opool", bufs=2))

        hT = persist.tile([P, NT, T], fp8, tag="hT")
        wk = persist.tile([P, NT, DG], fp8, tag="wk")
        wv = persist.tile([P, NT, DG], fp8, tag="wv")
        # chunked input DMAs: the first qkT matmuls only need the first chunks
        for j in range(4):
            nc.sync.dma_start(hT[:, 2 * j:2 * j + 2, :],
                              hTd[:, 2 * T * j:2 * T * (j + 1)])
            nc.sync.dma_start(wk[:, 2 * j:2 * j + 2, :],
                              wkd[:, 2 * DG * j:2 * DG * (j + 1)])
        nc.sync.dma_start(wv[:], wvd[:])
        tri = persist.tile([P, 2, P], fp8, tag="tri")
        nc.sync.dma_start(tri[:], trid[:])

        qkT = [persist.tile([P, T], bf16, tag=f"qkT{pr}", name=f"qkT{pr}")
               for pr in range(4)]
        v_aug = persist.tile([P, NT, NHG * VW], fp8, tag="vaug")
        nc.gpsimd.memset(v_aug[:], 1.0)
        eofft = persist.tile([P, 1], f32, tag="eofft")
        nc.vector.memset(eofft[:], EOFF)

        # HAM warmup: keep the PE busy on throwaway matmuls during the input
        # DMA ramp so the clock gate is at 8/8 when real matmuls start
        dum = persist.tile([P, TCH], bf16, tag="dum")
        nc.vector.memset(dum[:], 0.0)
        dps = psum.tile([P, TCH], f32, tag="mm", bufs=2, name="dps")
        for _ in range(36):
            nc.tensor.matmul(dps[:], dum[:, 0:P], dum[:],
                             start=True, stop=True)
        nc.vector.tensor_copy(dum[:], dps[:])

        def emit_qkT_h(pr, tj):
            ps = psum.tile([P, TCH], f32, tag="mm", bufs=2, name="ps")
            for j in range(4):
                nc.tensor.matmul(
                    ps[:],
                    wk[:, 2 * j:2 * j + 2, P * pr:P * (pr + 1)],
                    hT[:, 2 * j:2 * j + 2, TCH * tj:TCH * (tj + 1)],
                    start=(j == 0), stop=(j == 3), perf_mode=DR)
            # 2^-8 undoes the host weight pre-scale -> true qk in bf16
            nc.vector.tensor_scalar_mul(
                qkT[pr][:, TCH * tj:TCH * (tj + 1)], ps[:], 1.0 / WS1)

        def emit_v(si):
            ps = psum.tile([P, TCH], f32, tag="mm", bufs=2, name="ps")
            for j in range(4):
                nc.tensor.matmul(
                    ps[:],
                    hT[:, 2 * j:2 * j + 2, P * si:P * (si + 1)],
                    wv[:, 2 * j:2 * j + 2, :],
                    start=(j == 0), stop=(j == 3), perf_mode=DR)
            va = v_aug[:, si, :].rearrange("p (h c) -> p h c", c=VW)
            nc.vector.tensor_scalar_mul(
                va[:, :, 0:HD],
                ps[:].rearrange("p (h c) -> p h c", c=HD),
                1.0 / WS1)

        # p tiles: [s-part, slot(=8*tj+si), k(head), t-chunk] fp8 so every
        # chunk's byte range is contiguous -> exact dependency ranges
        avps = {}

        def emit_av_jp(pr, tj, jp, pT):
            """AV accumulation step jp (si pair 2jp,2jp+1) for both heads;
            the last step also evacuates + DMAs out."""
            npair = 2 * tj + 2
            if jp == 0:
                avps[pr, tj] = psum.tile([65, 2, TCH], f32, tag="av", bufs=1,
                                         name="avp")
            ps = avps[pr, tj]
            for k in range(2):
                h = 2 * pr + k
                nc.tensor.matmul(
                    ps[:, k, :],
                    v_aug[:, 2 * jp:2 * jp + 2, VW * h:VW * h + 65],
                    pT[:, 8 * tj + 2 * jp:8 * tj + 2 * jp + 2, k, :],
                    start=(jp == 0), stop=(jp == npair - 1), perf_mode=DR)
            if jp == npair - 1:
                ot = opool.tile([65, 2, TCH], bf16, tag="ot", name="ot")
                nc.vector.tensor_copy(ot[:], ps[:])
                nc.sync.dma_start(
                    avd[65 * pr:65 * pr + 65, T * tj:T * (tj + 1)], ot[:])

        def emit_scores(pr, tj, pT, acts):
            """Scores+exp+mask chunks for (pr, tj); after chunk i the
            closure acts[i] (if any) is emitted to keep the PE streaming.
            tj=1 groups additionally run their own AV pairs at lag 2."""
            tb = TCH * tj
            n = 4 * tj + 4
            for si in range(n):
                slot = 8 * tj + si
                ps = psum.tile([P, 2, TCH], f32, tag="sc", bufs=2, name="ps")
                for k in range(2):
                    off = HD * k
                    nc.tensor.matmul(
                        ps[:, k, :],
                        qkT[pr][off:off + HD, P * si:P * (si + 1)],
                        qkT[pr][off:off + HD, tb:tb + TCH],
                        start=True, stop=True, tile_position=(off, 0))
                m = si - 4 * tj
                if m <= 0:
                    nc.scalar.activation(pT[:, slot, :, 0:TCH], ps[:],
                                         Exp, bias=eofft[:], scale=SCALE)
                else:
                    # cols [0, 128m) were zeroed at tile allocation
                    nc.scalar.activation(pT[:, slot, :, P * m:TCH],
                                         ps[:, :, P * m:TCH],
                                         Exp, bias=eofft[:], scale=SCALE)
                if m >= 0:
                    eng = nc.gpsimd if m % 2 == 0 else nc.vector
                    eng.tensor_mul(pT[:, slot, :, P * m:P * (m + 1)],
                                   pT[:, slot, :, P * m:P * (m + 1)],
                                   tri[:])
                if tj == 1 and si >= 5 and si % 2 == 1:
                    emit_av_jp(pr, 1, (si - 5) // 2, pT)
                elif acts:
                    a = acts.pop(0)
                    if a is not None:
                        a()
            if tj == 1:
                emit_av_jp(pr, 1, 2, pT)
                emit_av_jp(pr, 1, 3, pT)

        pTs = {}

        def new_pT(pr):
            pT = ppool.tile([P, 2 * NT, 2, TCH], fp8, tag="pT", name=f"pT{pr}")
            pTs[pr] = pT
            # pre-zero the fully-masked column blocks of on-diagonal slots
            for tj in range(NTC):
                for m in range(1, 4):
                    nc.gpsimd.memset(
                        pT[:, 8 * tj + 4 * tj + m, :, 0:P * m], 0.0)
            return pT

        AV = emit_av_jp
        emit_qkT_h(0, 0)
        emit_v(0)
        emit_scores(0, 0, new_pT(0),
                    [lambda: emit_qkT_h(0, 1), lambda: emit_qkT_h(1, 0),
                     lambda: emit_qkT_h(1, 1), lambda: emit_v(1)])
        emit_scores(0, 1, pTs[0],
                    [lambda: emit_v(2), lambda: emit_v(3),
                     lambda: emit_v(4), lambda: emit_v(5),
                     lambda: emit_v(6), lambda: emit_v(7)])
        emit_scores(1, 0, new_pT(1),
                    [lambda: emit_qkT_h(2, 0), lambda: emit_qkT_h(2, 1),
                     lambda: AV(0, 0, 0, pTs[0]),
                     lambda: AV(0, 0, 1, pTs[0])])
        emit_scores(1, 1, pTs[1],
                    [lambda: emit_qkT_h(3, 0), lambda: emit_qkT_h(3, 1),
                     None, None, None, None])
        emit_scores(2, 0, new_pT(2),
                    [None, None, lambda: AV(1, 0, 0, pTs[1]),
                     lambda: AV(1, 0, 1, pTs[1])])
        emit_scores(2, 1, pTs[2], [None] * 6)
        # pr=3: tj=1 first so the final tail is the cheaper tj=0 AV
        emit_scores(3, 1, new_pT(3),
                    [None, None, lambda: AV(2, 0, 0, pTs[2]),
                     lambda: AV(2, 0, 1, pTs[2]), None, None])
        emit_scores(3, 0, pTs[3],
                    [None, None, lambda: AV(3, 0, 0, pTs[3]), None])
        emit_av_jp(3, 0, 1, pTs[3])

    nc.compile()
    return nc


def build_phase_b():
    nc = bacc.Bacc("TRN2", target_bir_lowering=False, debug=False)
    h2d = nc.dram_tensor("h2T", [P, NT * T], fp8, kind="ExternalInput")
    # w1c: fi-major blocks; block fi is [128, 8*128] with [p, 128*ci + q] =
    # W1T[128*ci + p, FG*g + 128*fi + q] * 2^8
    w1d = nc.dram_tensor("w1c", [P, NF * C], fp8, kind="ExternalInput")
    b1d = nc.dram_tensor("b1", [P, NF], f32, kind="ExternalInput")
    w2d = nc.dram_tensor("w2T", [P, NF * C], fp8, kind="ExternalInput")
    ffd = nc.dram_tensor("ffpT", [C, T], bf16, kind="ExternalOutput")

    with tile.TileContext(nc) as tc, ExitStack() as ctx:
        persist = ctx.enter_context(tc.tile_pool(name="persist", bufs=1))
        psum = ctx.enter_context(tc.tile_pool(name="psum", bufs=1, space="PSUM"))
        opool = ctx.enter_context(tc.tile_pool(name="opool", bufs=2))

        h2 = persist.tile([P, NT, T], fp8, tag="h2")
        w1 = persist.tile([P, NF, NT, P], fp8, tag="w1")
        b1 = persist.tile([P, NF], f32, tag="b1")
        # smallest/first-needed pieces first so fi=0 matmuls start early
        nc.sync.dma_start(b1[:], b1d[:])
        nc.sync.dma_start(h2[:, 0:2, :], h2d[:, 0:2 * T])
        nc.sync.dma_start(w1[:, 0:2, :, :], w1d[:, 0:2 * C])
        for j in range(1, 4):
            nc.sync.dma_start(h2[:, 2 * j:2 * j + 2, :],
                              h2d[:, 2 * T * j:2 * T * (j + 1)])
        nc.sync.dma_start(w1[:, 2:8, :, :], w1d[:, 2 * C:8 * C])
        nc.sync.dma_start(w1[:, 8:16, :, :], w1d[:, 8 * C:16 * C])
        w2 = persist.tile([P, NF, C], fp8, tag="w2")
        for j in range(4):
            nc.gpsimd.dma_start(w2[:, 4 * j:4 * (j + 1), :],
                                w2d[:, 4 * C * j:4 * C * (j + 1)])

        relu = persist.tile([P, NF, T], fp8, tag="relu")

        # HAM warmup (see phase A)
        dum = persist.tile([P, TCH], bf16, tag="dum")
        nc.vector.memset(dum[:], 0.0)
        dps = psum.tile([P, 2, TCH], f32, tag="ff", bufs=4, name="dps")
        for _ in range(36):
            nc.tensor.matmul(dps[:, 0, :], dum[:, 0:P], dum[:],
                             start=True, stop=True)
        nc.vector.tensor_copy(dum[:], dps[:, 0, :])

        for fi in range(NF):
            ps = psum.tile([P, 2, TCH], f32, tag="ff", bufs=4, name="ps")
            for tj in range(NTC):
                for j in range(4):
                    nc.tensor.matmul(
                        ps[:, tj, :],
                        w1[:, fi, 2 * j:2 * j + 2, :],
                        h2[:, 2 * j:2 * j + 2, TCH * tj:TCH * (tj + 1)],
                        start=(j == 0), stop=(j == 3), perf_mode=DR)
            # 2^-8 undoes the host W1 pre-scale before the nonlinearity;
            # per-tj so FFN2 can chase the last tile with minimal tail
            for tj in range(NTC):
                nc.scalar.activation(
                    relu[:, fi, TCH * tj:TCH * (tj + 1)], ps[:, tj, :],
                    Relu, bias=b1[:, fi:fi + 1], scale=1.0 / WS1)

        for c2 in range(NT):
            ps = psum.tile([P, 2, TCH], f32, tag="ff", bufs=4, name="ps")
            for tj in range(NTC):
                for j in range(NF // 2):
                    nc.tensor.matmul(
                        ps[:, tj, :],
                        w2[:, 2 * j:2 * j + 2, P * c2:P * (c2 + 1)],
                        relu[:, 2 * j:2 * j + 2, TCH * tj:TCH * (tj + 1)],
                        start=(j == 0), stop=(j == NF // 2 - 1), perf_mode=DR)
            ot = opool.tile([P, 2, TCH], bf16, tag="ot", name="ot")
            nc.vector.tensor_copy(ot[:], ps[:])
            nc.sync.dma_start(ffd[P * c2:P * (c2 + 1), :], ot[:])

    nc.compile()
    return nc


_CACHE = {}
TRACE = [False]
EXEC_NS = []


def _get_kernels():
    if "a" not in _CACHE:
        _CACHE["a"] = build_phase_a()
        _CACHE["b"] = build_phase_b()
    return _CACHE["a"], _CACHE["b"]


def _fp8(a):
    return np.clip(np.ascontiguousarray(a), -240, 240).astype(np_fp8)


def _sbufify(a):
    """[G*128, X] -> [128, G*X]: concatenate 128-row blocks along columns."""
    a = np.asarray(a)
    g = a.shape[0] // P
    return np.ascontiguousarray(
        a.reshape(g, P, a.shape[1]).transpose(1, 0, 2).reshape(P, -1))


def _tri2():
    """[128, 2*128] fp8: lower-triangular 0/1 mask (s<=t), twice (k=0,1)."""
    sl = np.arange(P)[:, None]
    tl = np.arange(P)[None, :]
    t1 = (sl <= tl).astype(np.float32)
    return np.concatenate([t1, t1], axis=1)


def _ln_host(x, gamma, beta):
    """LayerNorm over axis 0 of [T, C] with unbiased variance."""
    m = x.mean(axis=0, keepdims=True)
    v = x.var(axis=0, ddof=1, keepdims=True)
    g = np.asarray(gamma, np.float32)[None, :]
    bb = np.asarray(beta, np.float32)[None, :]
    return g * (x - m) / np.sqrt(v + EPS) + bb


def prep_a(ins, core):
    b, g = core // 2, core % 2
    heads = range(NHG * g, NHG * (g + 1))
    Wk = np.asarray(ins["Wk"], np.float32)
    Wv = np.asarray(ins["Wv"], np.float32)
    x = np.asarray(ins["x"], np.float32)
    h = _ln_host(x[b], ins["g1"], ins["beta1"])       # [T, C]
    return {
        "hT": _fp8(_sbufify(h.T)),
        "wk": _fp8(_sbufify(np.concatenate([Wk[h_] for h_ in heads], axis=1)) * WS1),
        "wv": _fp8(_sbufify(np.concatenate([Wv[h_] for h_ in heads], axis=1)) * WS1),
        "tri": _fp8(_tri2()),
    }


def attn_from_av(av):
    """av [260, 2048] bf16 -> normalized attn [T, 512] f32 for one core."""
    av = np.asarray(av, np.float32)
    attn = np.empty((T, DG), np.float32)
    for pr in range(4):
        blk = av[65 * pr:65 * pr + 65, :].reshape(65, 2, 2, TCH)  # [65,tj,k,t]
        for k in range(2):
            a = blk[0:64, :, k, :].reshape(64, T)                 # [64, t]
            d = blk[64, :, k, :].reshape(T)                       # [t]
            attn[:, HD * (2 * pr + k):HD * (2 * pr + k + 1)] = (a / d).T
    return attn


def _w1c_layout(W1T_g):
    """[C, FG] W1^T slice -> fi-major [128, NF*C] (see build_phase_b)."""
    out = np.empty((P, NF * C), np.float32)
    for fi in range(NF):
        blk = W1T_g[:, P * fi:P * (fi + 1)]          # [C, 128]
        out[:, C * fi:C * (fi + 1)] = _sbufify(blk)
    return out


def prep_b(ins, x2, core):
    b, g = core // 2, core % 2
    W1T_g = np.asarray(ins["W1"], np.float32).T[:, FG * g:FG * (g + 1)]
    h2 = _ln_host(x2[b], ins["g2"], ins["beta2"]).T   # [C, T]
    return {
        "h2T": _fp8(_sbufify(h2)),
        "w1c": _fp8(_w1c_layout(W1T_g) * WS1),
        "b1": np.ascontiguousarray(np.asarray(ins["b1"], np.float32)
                                   [FG * g:FG * (g + 1)].reshape(NF, P).T),
        "w2T": _fp8(_sbufify(np.asarray(ins["W2"], np.float32).T
                             [FG * g:FG * (g + 1), :]) * WS2),
    }


def kernel(x, Wk, Wv, Wp, bp, W1, b1, W2, b2, g1, beta1, g2, beta2):
    from concourse.bass_utils import run_bass_kernel_spmd

    ins = dict(x=x, Wk=Wk, Wv=Wv, Wp=Wp, bp=bp, W1=W1, b1=b1, W2=W2, b2=b2,
               g1=g1, beta1=beta1, g2=g2, beta2=beta2)
    nc_a, nc_b = _get_kernels()
    cores = list(range(8))
    x = np.asarray(x, dtype=np.float32)
    Wp = np.asarray(Wp, np.float32)

    # ---- Phase A ----
    in_maps_a = [prep_a(ins, c) for c in cores]
    ra = run_bass_kernel_spmd(nc_a, in_maps_a, cores, trace=TRACE[0])
    if TRACE[0]:
        EXEC_NS.append(ra.exec_time_ns)
        print("phase A exec_time_ns:", ra.exec_time_ns)
    res_a = ra.results

    x2 = np.empty_like(x)
    for b in range(B):
        acc = x[b] + np.asarray(bp, np.float32)[None, :]
        for g in range(2):
            attn = attn_from_av(res_a[2 * b + g]["av"])       # [T, 512]
            acc = acc + attn @ Wp[:, DG * g:DG * (g + 1)].T
        x2[b] = acc

    # ---- Phase B ----
    in_maps_b = [prep_b(ins, x2, c) for c in cores]
    rb = run_bass_kernel_spmd(nc_b, in_maps_b, cores, trace=TRACE[0])
    if TRACE[0]:
        EXEC_NS.append(rb.exec_time_ns)
        print("phase B exec_time_ns:", rb.exec_time_ns)
    res_b = rb.results

    out = np.empty_like(x)
    for b in range(B):
        out[b] = (x2[b]
                  + (res_b[2 * b]["ffpT"].astype(np.float32).T
                     + res_b[2 * b + 1]["ffpT"].astype(np.float32).T) / WS2
                  + np.asarray(b2, np.float32)[None, :])
    return out


# hooks for test.py: per-core numpy input prep used by the CoreSim path
def sim_feed_a(sim, ins, core):
    for k, v in prep_a(ins, core).items():
        sim.tensor(k)[:] = v


def sim_feed_b(sim, ins, x2, core):
    for k, v in prep_b(ins, x2, core).items():
        sim.tensor(k)[:] = v


# revision 23
# speedup vs baseline: 1.0087x; 1.0087x over previous
"""Trainium2 Bass kernel for a dense transformer block (B=4, T=1024, C=1024, H=16).

Sharding: 8 cores = 4 batches x 2 tensor-parallel groups (8 heads + half the
FFN hidden dim per core).

Phase A (attention, per core): takes h = LN1(x) (computed on host) in fp8
[C, T] layout.  On chip: qk/v head projections (fp8 DoubleRow matmuls,
K=256/instr), causal scores (bf16, two heads packed in the PE array via
tile_position), exp with a constant -3 offset folded in (keeps e4m3 in range;
cancels in the softmax ratio), and p@[v|1] fp8 DoubleRow matmuls whose extra
ones-column yields the softmax denominator for free.  Ships unnormalized
[attn ; den] per (head, t-chunk) to the host, which divides, applies the
output projection Wp, and forms x2 = x + attn@Wp^T + bp.  Score chunks are
round-robin interleaved with the other matmuls so the PE keeps streaming
while the Scalar engine (exp) drains behind it.

Phase B (FFN, per core): takes h2 = LN2(x2) (host) in fp8.  FFN1 (fp8
DoubleRow) -> ReLU (descale 2^-8 folded into the activation) -> FFN2 (fp8
DoubleRow) -> partial [C, T] output; host descales 2^-9, adds residual + b2.

Weights are pre-scaled by 2^8 (Wk, Wv, W1) / 2^9 (W2) on the host so their
uniform(-1/32..) ranges land in e4m3's normal range; all scales are powers of
two (exact) and are undone on-chip (ReLU) or on the host.
"""
import sys

sys.path.insert(0, "/opt/trn_rl_repo")

import numpy as np
import ml_dtypes
from contextlib import ExitStack

import concourse.bacc as bacc
import concourse.mybir as mybir
import concourse.tile as tile

bf16 = mybir.dt.bfloat16
f32 = mybir.dt.float32
fp8 = mybir.dt.float8e4
np_fp8 = ml_dtypes.float8_e4m3
DR = mybir.MatmulPerfMode.DoubleRow

B, T, C, H = 4, 1024, 1024, 16
HD = 64                    # head dim
NHG = 8                    # heads per core (group)
DG = NHG * HD              # 512, channel span per head group
F = 4 * C                  # 4096 FFN hidden
FG = F // 2                # 2048 per core
P = 128                    # partitions
EPS = 1e-5
SCALE = HD ** -0.5         # 0.125
EOFF = -3.0                # constant exp offset; cancels in softmax ratio

NT = T // P                # 8 c-tiles of 128
TCH = 512                  # t-chunk
NTC = T // TCH             # 2 t-chunks
NF = FG // P               # 16 hidden tiles per core
VW = 80                    # per-head stride in v_aug (65 used, padded to 80 for
                           # 16B-aligned DoubleRow slab steps)

WS1 = 2.0 ** 8             # host pre-scale on Wk/Wv/W1
WS2 = 2.0 ** 9             # host pre-scale on W2
Exp = mybir.ActivationFunctionType.Exp
Relu = mybir.ActivationFunctionType.Relu


def build_phase_a():
    nc = bacc.Bacc("TRN2", target_bir_lowering=False, debug=False)
    hTd = nc.dram_tensor("hT", [P, NT * T], fp8, kind="ExternalInput")
    wkd = nc.dram_tensor("wk", [P, NT * DG], fp8, kind="ExternalInput")
    wvd = nc.dram_tensor("wv", [P, NT * DG], fp8, kind="ExternalInput")
    trid = nc.dram_tensor("tri", [P, 2 * P], fp8, kind="ExternalInput")
    # av: per (pr, tj): rows [65*pr, 65*pr+65), cols [1024*tj + 512*k + t]
    # rows 0-63 = unnormalized attn dims of head 2*pr+k, row 64 = denominator
    avd = nc.dram_tensor("av", [4 * 65, 2 * T], bf16, kind="ExternalOutput")

    with tile.TileContext(nc) as tc, ExitStack() as ctx:
        persist = ctx.enter_context(tc.tile_pool(name="persist", bufs=1))
        psum = ctx.enter_context(tc.tile_pool(name="psum", bufs=1, space="PSUM"))
        ppool = ctx.enter_context(tc.tile_pool(name="ppool", bufs=2))
        opool = ctx.enter_context(tc.tile_pool(name="opool", bufs=2))

        hT = persist.tile([P, NT, T], fp8, tag="hT")
        wk = persist.tile([P, NT, DG], fp8, tag="wk")
        wv = persist.tile([P, NT, DG], fp8, tag="wv")
        # chunked input DMAs: the first qkT matmuls only need the first chunks
        for j in range(4):
            nc.sync.dma_start(hT[:, 2 * j:2 * j + 2, :],
                              hTd[:, 2 * T * j:2 * T * (j + 1)])
            nc.sync.dma_start(wk[:, 2 * j:2 * j + 2, :],
                              wkd[:, 2 * DG * j:2 * DG * (j + 1)])
        nc.sync.dma_start(wv[:], wvd[:])
        tri = persist.tile([P, 2, P], fp8, tag="tri")
        nc.sync.dma_start(tri[:], trid[:])

        qkT = [persist.tile([P, T], bf16, tag=f"qkT{pr}", name=f"qkT{pr}")
               for pr in range(4)]
        v_aug = persist.tile([P, NT, NHG * VW], fp8, tag="vaug")
        nc.gpsimd.memset(v_aug[:], 1.0)
        eofft = persist.tile([P, 1], f32, tag="eofft")
        nc.vector.memset(eofft[:], EOFF)


        def emit_qkT_h(pr, tj):
            ps = psum.tile([P, TCH], f32, tag="mm", bufs=2, name="ps")
            for j in range(4):
                nc.tensor.matmul(
                    ps[:],
                    wk[:, 2 * j:2 * j + 2, P * pr:P * (pr + 1)],
                    hT[:, 2 * j:2 * j + 2, TCH * tj:TCH * (tj + 1)],
                    start=(j == 0), stop=(j == 3), perf_mode=DR)
            # 2^-8 undoes the host weight pre-scale -> true qk in bf16
            nc.vector.tensor_scalar_mul(
                qkT[pr][:, TCH * tj:TCH * (tj + 1)], ps[:], 1.0 / WS1)

        def emit_v(si):
            ps = psum.tile([P, TCH], f32, tag="mm", bufs=2, name="ps")
            for j in range(4):
                nc.tensor.matmul(
                    ps[:],
                    hT[:, 2 * j:2 * j + 2, P * si:P * (si + 1)],
                    wv[:, 2 * j:2 * j + 2, :],
                    start=(j == 0), stop=(j == 3), perf_mode=DR)
            va = v_aug[:, si, :].rearrange("p (h c) -> p h c", c=VW)
            nc.vector.tensor_scalar_mul(
                va[:, :, 0:HD],
                ps[:].rearrange("p (h c) -> p h c", c=HD),
                1.0 / WS1)

        # p tiles: [s-part, slot(=8*tj+si), k(head), t-chunk] fp8 so every
        # chunk's byte range is contiguous -> exact dependency ranges
        avps = {}

        def emit_av_jp(pr, tj, jp, pT):
            """AV accumulation step jp (si pair 2jp,2jp+1) for both heads;
            the last step also evacuates + DMAs out."""
            npair = 2 * tj + 2
            if jp == 0:
                avps[pr, tj] = psum.tile([65, 2, TCH], f32, tag="av", bufs=1,
                                         name="avp")
            ps = avps[pr, tj]
            for k in range(2):
                h = 2 * pr + k
                nc.tensor.matmul(
                    ps[:, k, :],
                    v_aug[:, 2 * jp:2 * jp + 2, VW * h:VW * h + 65],
                    pT[:, 8 * tj + 2 * jp:8 * tj + 2 * jp + 2, k, :],
                    start=(jp == 0), stop=(jp == npair - 1), perf_mode=DR)
            if jp == npair - 1:
                ot = opool.tile([65, 2, TCH], bf16, tag="ot", name="ot")
                nc.vector.tensor_copy(ot[:], ps[:])
                nc.sync.dma_start(
                    avd[65 * pr:65 * pr + 65, T * tj:T * (tj + 1)], ot[:])

        def emit_scores(pr, tj, pT, acts):
            """Scores+exp+mask chunks for (pr, tj); after chunk i the
            closure acts[i] (if any) is emitted to keep the PE streaming.
            tj=1 groups additionally run their own AV pairs at lag 2."""
            tb = TCH * tj
            n = 4 * tj + 4
            for si in range(n):
                slot = 8 * tj + si
                ps = psum.tile([P, 2, TCH], f32, tag="sc", bufs=2, name="ps")
                for k in range(2):
                    off = HD * k
                    nc.tensor.matmul(
                        ps[:, k, :],
                        qkT[pr][off:off + HD, P * si:P * (si + 1)],
                        qkT[pr][off:off + HD, tb:tb + TCH],
                        start=True, stop=True, tile_position=(off, 0))
                m = si - 4 * tj
                if m <= 0:
                    nc.scalar.activation(pT[:, slot, :, 0:TCH], ps[:],
                                         Exp, bias=eofft[:], scale=SCALE)
                else:
                    # cols [0, 128m) were zeroed at tile allocation
                    nc.scalar.activation(pT[:, slot, :, P * m:TCH],
                                         ps[:, :, P * m:TCH],
                                         Exp, bias=eofft[:], scale=SCALE)
                if m >= 0:
                    eng = nc.gpsimd if m % 2 == 0 else nc.vector
                    eng.tensor_mul(pT[:, slot, :, P * m:P * (m + 1)],
                                   pT[:, slot, :, P * m:P * (m + 1)],
                                   tri[:])
                if tj == 1 and si >= 5 and si % 2 == 1:
                    emit_av_jp(pr, 1, (si - 5) // 2, pT)
                elif acts:
                    a = acts.pop(0)
                    if a is not None:
                        a()
            if tj == 1:
                emit_av_jp(pr, 1, 2, pT)
                emit_av_jp(pr, 1, 3, pT)

        pTs = {}

        def new_pT(pr):
            pT = ppool.tile([P, 2 * NT, 2, TCH], fp8, tag="pT", name=f"pT{pr}")
            pTs[pr] = pT
            # pre-zero the fully-masked column blocks of on-diagonal slots
            for tj in range(NTC):
                for m in range(1, 4):
                    nc.gpsimd.memset(
                        pT[:, 8 * tj + 4 * tj + m, :, 0:P * m], 0.0)
            return pT

        AV = emit_av_jp
        emit_qkT_h(0, 0)
        emit_v(0)
        emit_scores(0, 0, new_pT(0),
                    [lambda: emit_qkT_h(0, 1), lambda: emit_qkT_h(1, 0),
                     lambda: emit_qkT_h(1, 1), lambda: emit_v(1)])
        emit_scores(0, 1, pTs[0],
                    [lambda: emit_v(2), lambda: emit_v(3),
                     lambda: emit_v(4), lambda: emit_v(5),
                     lambda: emit_v(6), lambda: emit_v(7)])
        emit_scores(1, 0, new_pT(1),
                    [lambda: emit_qkT_h(2, 0), lambda: emit_qkT_h(2, 1),
                     lambda: AV(0, 0, 0, pTs[0]),
                     lambda: AV(0, 0, 1, pTs[0])])
        emit_scores(1, 1, pTs[1],
                    [lambda: emit_qkT_h(3, 0), lambda: emit_qkT_h(3, 1),
                     None, None, None, None])
        emit_scores(2, 0, new_pT(2),
                    [None, None, lambda: AV(1, 0, 0, pTs[1]),
                     lambda: AV(1, 0, 1, pTs[1])])
        emit_scores(2, 1, pTs[2], [None] * 6)
        # pr=3: tj=1 first so the final tail is the cheaper tj=0 AV
        emit_scores(3, 1, new_pT(3),
                    [None, None, lambda: AV(2, 0, 0, pTs[2]),
                     lambda: AV(2, 0, 1, pTs[2]), None, None])
        emit_scores(3, 0, pTs[3],
                    [None, None, lambda: AV(3, 0, 0, pTs[3]), None])
        emit_av_jp(3, 0, 1, pTs[3])

    nc.compile()
    return nc


def build_phase_b():
    nc = bacc.Bacc("TRN2", target_bir_lowering=False, debug=False)
    h2d = nc.dram_tensor("h2T", [P, NT * T], fp8, kind="ExternalInput")
    # w1c: fi-major blocks; block fi is [128, 8*128] with [p, 128*ci + q] =
    # W1T[128*ci + p, FG*g + 128*fi + q] * 2^8
    w1d = nc.dram_tensor("w1c", [P, NF * C], fp8, kind="ExternalInput")
    b1d = nc.dram_tensor("b1", [P, NF], f32, kind="ExternalInput")
    w2d = nc.dram_tensor("w2T", [P, NF * C], fp8, kind="ExternalInput")
    ffd = nc.dram_tensor("ffpT", [C, T], bf16, kind="ExternalOutput")

    with tile.TileContext(nc) as tc, ExitStack() as ctx:
        persist = ctx.enter_context(tc.tile_pool(name="persist", bufs=1))
        psum = ctx.enter_context(tc.tile_pool(name="psum", bufs=1, space="PSUM"))
        opool = ctx.enter_context(tc.tile_pool(name="opool", bufs=2))

        h2 = persist.tile([P, NT, T], fp8, tag="h2")
        w1 = persist.tile([P, NF, NT, P], fp8, tag="w1")
        b1 = persist.tile([P, NF], f32, tag="b1")
        # smallest/first-needed pieces first so fi=0 matmuls start early
        nc.sync.dma_start(b1[:], b1d[:])
        nc.sync.dma_start(h2[:, 0:2, :], h2d[:, 0:2 * T])
        nc.sync.dma_start(w1[:, 0:2, :, :], w1d[:, 0:2 * C])
        for j in range(1, 4):
            nc.sync.dma_start(h2[:, 2 * j:2 * j + 2, :],
                              h2d[:, 2 * T * j:2 * T * (j + 1)])
        nc.sync.dma_start(w1[:, 2:8, :, :], w1d[:, 2 * C:8 * C])
        nc.sync.dma_start(w1[:, 8:16, :, :], w1d[:, 8 * C:16 * C])
        w2 = persist.tile([P, NF, C], fp8, tag="w2")
        for j in range(4):
            nc.gpsimd.dma_start(w2[:, 4 * j:4 * (j + 1), :],
                                w2d[:, 4 * C * j:4 * C * (j + 1)])

        relu = persist.tile([P, NF, T], fp8, tag="relu")


        for fi in range(NF):
            ps = psum.tile([P, 2, TCH], f32, tag="ff", bufs=4, name="ps")
            for tj in range(NTC):
                for j in range(4):
                    nc.tensor.matmul(
                        ps[:, tj, :],
                        w1[:, fi, 2 * j:2 * j + 2, :],
                        h2[:, 2 * j:2 * j + 2, TCH * tj:TCH * (tj + 1)],
                        start=(j == 0), stop=(j == 3), perf_mode=DR)
            # 2^-8 undoes the host W1 pre-scale before the nonlinearity;
            # per-tj so FFN2 can chase the last tile with minimal tail
            for tj in range(NTC):
                nc.scalar.activation(
                    relu[:, fi, TCH * tj:TCH * (tj + 1)], ps[:, tj, :],
                    Relu, bias=b1[:, fi:fi + 1], scale=1.0 / WS1)

        for c2 in range(NT):
            ps = psum.tile([P, 2, TCH], f32, tag="ff", bufs=4, name="ps")
            for tj in range(NTC):
                for j in range(NF // 2):
                    nc.tensor.matmul(
                        ps[:, tj, :],
                        w2[:, 2 * j:2 * j + 2, P * c2:P * (c2 + 1)],
                        relu[:, 2 * j:2 * j + 2, TCH * tj:TCH * (tj + 1)],
                        start=(j == 0), stop=(j == NF // 2 - 1), perf_mode=DR)
            ot = opool.tile([P, 2, TCH], bf16, tag="ot", name="ot")
            nc.vector.tensor_copy(ot[:], ps[:])
            nc.sync.dma_start(ffd[P * c2:P * (c2 + 1), :], ot[:])

    nc.compile()
    return nc


_CACHE = {}
TRACE = [False]
EXEC_NS = []


def _get_kernels():
    if "a" not in _CACHE:
        _CACHE["a"] = build_phase_a()
        _CACHE["b"] = build_phase_b()
    return _CACHE["a"], _CACHE["b"]


def _fp8(a):
    return np.clip(np.ascontiguousarray(a), -240, 240).astype(np_fp8)


def _sbufify(a):
    """[G*128, X] -> [128, G*X]: concatenate 128-row blocks along columns."""
    a = np.asarray(a)
    g = a.shape[0] // P
    return np.ascontiguousarray(
        a.reshape(g, P, a.shape[1]).transpose(1, 0, 2).reshape(P, -1))


def _tri2():
    """[128, 2*128] fp8: lower-triangular 0/1 mask (s<=t), twice (k=0,1)."""
    sl = np.arange(P)[:, None]
    tl = np.arange(P)[None, :]
    t1 = (sl <= tl).astype(np.float32)
    return np.concatenate([t1, t1], axis=1)


def _ln_host(x, gamma, beta):
    """LayerNorm over axis 0 of [T, C] with unbiased variance."""
    m = x.mean(axis=0, keepdims=True)
    v = x.var(axis=0, ddof=1, keepdims=True)
    g = np.asarray(gamma, np.float32)[None, :]
    bb = np.asarray(beta, np.float32)[None, :]
    return g * (x - m) / np.sqrt(v + EPS) + bb


def prep_a(ins, core):
    b, g = core // 2, core % 2
    heads = range(NHG * g, NHG * (g + 1))
    Wk = np.asarray(ins["Wk"], np.float32)
    Wv = np.asarray(ins["Wv"], np.float32)
    x = np.asarray(ins["x"], np.float32)
    h = _ln_host(x[b], ins["g1"], ins["beta1"])       # [T, C]
    return {
        "hT": _fp8(_sbufify(h.T)),
        "wk": _fp8(_sbufify(np.concatenate([Wk[h_] for h_ in heads], axis=1)) * WS1),
        "wv": _fp8(_sbufify(np.concatenate([Wv[h_] for h_ in heads], axis=1)) * WS1),
        "tri": _fp8(_tri2()),
    }


def attn_from_av(av):
    """av [260, 2048] bf16 -> normalized attn [T, 512] f32 for one core."""
    av = np.asarray(av, np.float32)
    attn = np.empty((T, DG), np.float32)
    for pr in range(4):
        blk = av[65 * pr:65 * pr + 65, :].reshape(65, 2, 2, TCH)  # [65,tj,k,t]
        for k in range(2):
            a = blk[0:64, :, k, :].reshape(64, T)                 # [64, t]
            d = blk[64, :, k, :].reshape(T)                       # [t]
            attn[:, HD * (2 * pr + k):HD * (2 * pr + k + 1)] = (a / d).T
    return attn


def _w1c_layout(W1T_g):
    """[C, FG] W1^T slice -> fi-major [128, NF*C] (see build_phase_b)."""
    out = np.empty((P, NF * C), np.float32)
    for fi in range(NF):
        blk = W1T_g[:, P * fi:P * (fi + 1)]          # [C, 128]
        out[:, C * fi:C * (fi + 1)] = _sbufify(blk)
    return out


def prep_b(ins, x2, core):
    b, g = core // 2, core % 2
    W1T_g = np.asarray(ins["W1"], np.float32).T[:, FG * g:FG * (g + 1)]
    h2 = _ln_host(x2[b], ins["g2"], ins["beta2"]).T   # [C, T]
    return {
        "h2T": _fp8(_sbufify(h2)),
        "w1c": _fp8(_w1c_layout(W1T_g) * WS1),
        "b1": np.ascontiguousarray(np.asarray(ins["b1"], np.float32)
                                   [FG * g:FG * (g + 1)].reshape(NF, P).T),
        "w2T": _fp8(_sbufify(np.asarray(ins["W2"], np.float32).T
                             [FG * g:FG * (g + 1), :]) * WS2),
    }


def kernel(x, Wk, Wv, Wp, bp, W1, b1, W2, b2, g1, beta1, g2, beta2):
    from concourse.bass_utils import run_bass_kernel_spmd

    ins = dict(x=x, Wk=Wk, Wv=Wv, Wp=Wp, bp=bp, W1=W1, b1=b1, W2=W2, b2=b2,
               g1=g1, beta1=beta1, g2=g2, beta2=beta2)
    nc_a, nc_b = _get_kernels()
    cores = list(range(8))
    x = np.asarray(x, dtype=np.float32)
    Wp = np.asarray(Wp, np.float32)

    # ---- Phase A ----
    in_maps_a = [prep_a(ins, c) for c in cores]
    ra = run_bass_kernel_spmd(nc_a, in_maps_a, cores, trace=TRACE[0])
    if TRACE[0]:
        EXEC_NS.append(ra.exec_time_ns)
        print("phase A exec_time_ns:", ra.exec_time_ns)
    res_a = ra.results

    x2 = np.empty_like(x)
    for b in range(B):
        acc = x[b] + np.asarray(bp, np.float32)[None, :]
        for g in range(2):
            attn = attn_from_av(res_a[2 * b + g]["av"])       # [T, 512]
            acc = acc + attn @ Wp[:, DG * g:DG * (g + 1)].T
        x2[b] = acc

    # ---- Phase B ----
    in_maps_b = [prep_b(ins, x2, c) for c in cores]
    rb = run_bass_kernel_spmd(nc_b, in_maps_b, cores, trace=TRACE[0])
    if TRACE[0]:
        EXEC_NS.append(rb.exec_time_ns)
        print("phase B exec_time_ns:", rb.exec_time_ns)
    res_b = rb.results

    out = np.empty_like(x)
    for b in range(B):
        out[b] = (x2[b]
                  + (res_b[2 * b]["ffpT"].astype(np.float32).T
                     + res_b[2 * b + 1]["ffpT"].astype(np.float32).T) / WS2
                  + np.asarray(b2, np.float32)[None, :])
    return out


# hooks for test.py: per-core numpy input prep used by the CoreSim path
def sim_feed_a(sim, ins, core):
    for k, v in prep_a(ins, core).items():
        sim.tensor(k)[:] = v


def sim_feed_b(sim, ins, x2, core):
    for k, v in prep_b(ins, x2, core).items():
        sim.tensor(k)[:] = v
